# revision 15
# baseline (speedup 1.0000x reference)
"""Depthwise 5x5 box filter (stride 1, 'same' zero padding) on TRN2.

Input x: (16, 8, 512, 512) f32, weight: (1, 1, 5, 5) f32 (uniform box kernel).
Output: (16, 8, 512, 512) f32.

Strategy (v2)
-------------
Data-parallel over the 128 independent (n, c) planes: 16 planes per core
across 8 cores.  Per plane, the separable 5-tap box filter runs on the
TensorEngine as two "transposing" banded matmuls (pass A vertical, pass B
horizontal); each pass contracts over the partition dim so two passes
restore the original orientation with no explicit transposes.

v2 changes vs v1 (62 us):

  * Input is shipped as fp8 e3m4 (PE-native dtype): host-side RNE cast of
    the f32 input.  Quantization rel-L2 ~1.34e-2 << 2e-2 budget; halves
    input HBM traffic (4.2 MB/core) and SBUF footprint.  Pass A runs
    fp8 x fp8 (img x 0/1 band, exact), pass B fp16 x fp16 as before.
  * PSUM->SBUF drain restructured: each pass accumulates into ONE
    4-bank [128, 2048] PSUM tile, drained by exactly TWO ops: ACT takes
    cols [0:XSPLIT], DVE takes [XSPLIT:2048] (XSPLIT=1088 balances
    ACT@1.2GHz+~260ns/op against DVE@0.96GHz+~150ns/op at ~2.3us/plane
    per engine -- v1's 4-single-on-ACT split ran ACT at 2.76us/plane).
    The straddling drains still free bank 0..2 early enough for the
    next plane's matmuls (pipeline period ~2.4us > drain-op 1.2us).
  * Steady walls per plane: drains ~2.3us (ACT and DVE each), PE ~2.2us,
    DMA (256KB in + 512KB out)/420GB/s ~1.8us.
  * Tail: the last plane's stores are issued from ACT (HWDGE, right
    after its own B-drain), gpsimd and sync in parallel, in quarters,
    instead of 4 serialized ~650ns gpsimd issues.

Engine layout: PE interleaves pass A of plane p with pass B of plane p-1
(software pipeline, LAG=1).  32+8 warm-up matmuls lift the HAM clock gate
(1.2 -> 2.4 GHz) during the framework preamble's dead window.
"""

import os
from contextlib import ExitStack

import ml_dtypes
import numpy as np

import concourse.bacc as bacc
import concourse.tile as tile
from concourse import mybir
from concourse.bass_utils import run_bass_kernel_spmd

N_CORES = 8
PLANES_TOTAL = 128  # 16 batch * 8 channels
PLANES_PER_CORE = PLANES_TOTAL // N_CORES  # 16
H = W = 512
P = 128  # partitions / K-block
NB = P + 4  # band matrix columns
KTAP = 5
KPAD = 2

USE_FP8 = os.environ.get("BOXF_FP8", "1") == "1"
# PSUM drain split: ACT takes banks 0-1 (cols 0:1024) of each pass, DVE
# banks 2-3.  Must be (a) bank-aligned (ScalarE+VectorE may not touch the
# same PSUM bank concurrently) and (b) SEPARATE TILES (the tile framework
# serializes two engine-readers of one PSUM tile even on disjoint banks).
XSPLIT = 2 * W

MM_DT = mybir.dt.float16
NP_IO_DT = np.float16
A_DT = mybir.dt.float8e3 if USE_FP8 else mybir.dt.float16
NP_A_DT = ml_dtypes.float8_e3m4 if USE_FP8 else np.float16

# Per PSUM bank (one 512-wide output window) the 4 K-block matmuls write
# overlapping band windows; the first (start=True) clears the whole-bank
# pending-zero region, and subsequent matmuls accumulate where written /
# overwrite where pending, per-element (PSUM has_written semantics).
# (kb, out_lo, out_hi, band_lo, band_hi, start)
BANK_PLAN = [
    (0, 0, 130, 2, 132, True),
    (1, 126, 258, 0, 132, False),
    (2, 254, 386, 0, 132, False),
    (3, 382, 512, 0, 130, False),
]


def _band_host(np_dt) -> np.ndarray:
    """B[p, j] = 1.0 iff 0 <= j - p <= 4, shape [128, 132]."""
    b = np.zeros((P, NB), dtype=np.float32)
    for p in range(P):
        b[p, p : p + KTAP] = 1.0
    return b.astype(np_dt)


def _emit_bank(nc, ps_bank, band, lhsT_of, last_bank):
    for i, (kb, o0, o1, b0, b1, start) in enumerate(BANK_PLAN):
        nc.tensor.matmul(
            ps_bank[:, o0:o1],
            lhsT_of(kb),
            band[:, b0:b1],
            start=start,
            stop=(last_bank and i == len(BANK_PLAN) - 1),
        )


def _build_nc(scale: float):
    nc = bacc.Bacc("TRN2", num_devices=N_CORES, num_swdge_queues=4)
    # xs/ys live in DRAM pre-swizzled by the host to match the SBUF
    # partition-line layout exactly: element [pl, p, kb*W + w] is plane
    # pl's pixel (row kb*128 + p, col w).  Each partition line is one
    # contiguous DRAM chunk (2 KB fp8 in / 4 KB fp16 out) so every DMA
    # descriptor is maximal.
    # xs is stored as 4 groups of 4 planes, group-major then partition:
    # xs[g][p, k*2048 + c] is plane (4g+k)'s partition-p line.  Groups
    # 1..3 load with ONE 1 MB DMA each (8 KB per partition line).
    xs = nc.declare_dram_parameter(
        "xs", [4, P, 4 * 4 * W], A_DT, isOutput=False
    )
    banda_d = nc.declare_dram_parameter("banda", [P, NB], A_DT, isOutput=False)
    bandb_d = nc.declare_dram_parameter("bandb", [P, NB], MM_DT, isOutput=False)
    ys = nc.declare_dram_parameter(
        "ys", [PLANES_PER_CORE, P, 4 * W], MM_DT, isOutput=True
    )

    with ExitStack() as ctx:
        tc = ctx.enter_context(tile.TileContext(nc))
        const_pool = ctx.enter_context(tc.tile_pool(name="const", bufs=1))
        img_pool = ctx.enter_context(tc.tile_pool(name="img", bufs=4))
        gimg_pool = ctx.enter_context(tc.tile_pool(name="gimg", bufs=3))
        # 10-deep mid/out rotation: shallow pools put plane p's drains
        # behind plane p-k's consumers (cross-engine WAR stalls).
        mid_pool = ctx.enter_context(tc.tile_pool(name="mid", bufs=10))
        out_pool = ctx.enter_context(tc.tile_pool(name="out", bufs=10))
        psa_lo_pool = ctx.enter_context(tc.tile_pool(name="psal", bufs=1, space="PSUM"))
        psa_hi_pool = ctx.enter_context(tc.tile_pool(name="psah", bufs=1, space="PSUM"))
        psb_lo_pool = ctx.enter_context(tc.tile_pool(name="psbl", bufs=1, space="PSUM"))
        psb_hi_pool = ctx.enter_context(tc.tile_pool(name="psbh", bufs=1, space="PSUM"))

        banda = const_pool.tile([P, NB], A_DT, tag="banda")
        bandb = const_pool.tile([P, NB], MM_DT, tag="bandb")
        # Band must be the first Sync issue: on the ACT ring it queues
        # behind the auto-inserted ACT_TABLE_LOAD and delays every
        # pass-A matmul by ~2 us.
        nc.sync.dma_start(banda[:], banda_d[:])
        nc.sync.dma_start(bandb[:], bandb_d[:])

        # PE warm-up: the HAM clock gate holds the PE at 1.2 GHz until
        # it has been busy for a ~3.4 us activity window.  The first
        # input's DMA completion lands ~3.5 us after the preamble ends,
        # so burn that dead window on dummy matmuls over a memset
        # scratch tile -- the first real pass then runs at 2.4 GHz.
        warm_src = const_pool.tile([P, P], MM_DT, tag="warm")
        nc.gpsimd.memset(warm_src[:], 0)
        warm_ps = psa_lo_pool.tile(
            [P, 2 * W], mybir.dt.float32, tag="psal", name="warm"
        )
        for _ in range(32):
            nc.tensor.matmul(
                warm_ps[:, 0:P], warm_src[:], warm_src[:], start=True, stop=True
            )

        # All input DMAs up-front: the Sync/HWDGE ring issues them
        # back-to-back so the input stream saturates HBM from the start.
        # SBUF holds all 16 fp8 planes (32 KB/partition).  Plane 0 is
        # split (quarter + quarter + half) so the first pass-A matmul
        # gates on a 64 KB piece; planes 4..15 are grouped 4-at-a-time
        # into one DMA each (fewer issue slots and completion
        # semaphores; their data still arrives well ahead of use).
        imgs = {}  # pl -> (tile, base column)
        for pl in range(4):
            img = img_pool.tile([P, 4 * W], A_DT, tag="img", name=f"img{pl}")
            if pl == 0:
                for c0, c1 in ((0, W), (W, 2 * W), (2 * W, 4 * W)):
                    nc.sync.dma_start(img[:, c0:c1], xs[0][:, c0:c1])
            else:
                b = pl * 4 * W
                nc.sync.dma_start(img[:], xs[0][:, b : b + 4 * W])
            imgs[pl] = (img, 0)
        for g in range(1, 4):
            gimg = gimg_pool.tile(
                [P, 4 * 4 * W], A_DT, tag="gimg", name=f"img_g{g}"
            )
            nc.sync.dma_start(gimg[:], xs[g])
            for k in range(4):
                imgs[4 * g + k] = (gimg, k * 4 * W)

        # Software pipeline, LAG=1: PE runs pass A of plane pl then pass
        # B of plane pl-1.  Each pass accumulates into one 4-bank PSUM
        # tile, drained by exactly two ops (ACT cols [0:XSPLIT], DVE the
        # rest) -- minimal per-op overhead at balanced engine load.
        LAG = 1
        mids, outs = {}, {}
        for pl in range(PLANES_PER_CORE + LAG):
            bp = pl - LAG
            last_plane = bp == PLANES_PER_CORE - 1
            if pl < PLANES_PER_CORE:
                psa_lo = psa_lo_pool.tile(
                    [P, 2 * W], mybir.dt.float32, tag="psal", name=f"psal{pl}"
                )
                psa_hi = psa_hi_pool.tile(
                    [P, 2 * W], mybir.dt.float32, tag="psah", name=f"psah{pl}"
                )
                img, ib = imgs[pl]
                for wb in range(4):
                    ps = psa_lo if wb < 2 else psa_hi
                    o = (wb % 2) * W
                    _emit_bank(
                        nc,
                        ps[:, o : o + W],
                        banda,
                        lambda kb, wb=wb: img[
                            :, ib + kb * W + wb * P : ib + kb * W + (wb + 1) * P
                        ],
                        last_bank=(wb % 2 == 1),
                    )
                mids[pl] = mid_pool.tile([P, 4 * W], MM_DT, tag="mid", name=f"mid{pl}")
                # Pass-A drain: plain downcast copies, ACT lo / DVE hi.
                nc.scalar.copy(mids[pl][:, 0:XSPLIT], psa_lo[:])
                nc.vector.tensor_copy(mids[pl][:, XSPLIT:], psa_hi[:])
            if pl == 0:
                # Second warm-up burst: fills the PE idle while the
                # first input's receipt lands, keeping the HAM activity
                # window busy through the pipeline fill.  Targets the
                # psb tile, which B(0) overwrites (start=True) after.
                fill_ps = psb_lo_pool.tile(
                    [P, 2 * W], mybir.dt.float32, tag="psbl", name="warmfill"
                )
                for _ in range(8):
                    nc.tensor.matmul(
                        fill_ps[:, 0:P], warm_src[:], warm_src[:],
                        start=True, stop=True,
                    )
            if bp >= 0:
                psb_lo = psb_lo_pool.tile(
                    [P, 2 * W], mybir.dt.float32, tag="psbl", name=f"psbl{bp}"
                )
                psb_hi = psb_hi_pool.tile(
                    [P, 2 * W], mybir.dt.float32, tag="psbh", name=f"psbh{bp}"
                )
                outs[bp] = out_pool.tile(
                    [P, 4 * W], MM_DT, tag="out", name=f"out{bp}"
                )
                mid = mids[bp]
                for wb in range(4):
                    ps = psb_lo if wb < 2 else psb_hi
                    o = (wb % 2) * W
                    _emit_bank(
                        nc,
                        ps[:, o : o + W],
                        bandb,
                        lambda kb, wb=wb: mid[
                            :, kb * W + wb * P : kb * W + (wb + 1) * P
                        ],
                        last_bank=(wb % 2 == 1),
                    )
                # Pass-B drain: fold the 1/25 scale into the downcast.
                nc.scalar.mul(outs[bp][:, 0:XSPLIT], psb_lo[:], scale)
                nc.vector.tensor_scalar_mul(
                    outs[bp][:, XSPLIT:], psb_hi[:], scale
                )
                if not last_plane:
                    # One full-plane output DMA on SWDGE (waits both
                    # drains via region deps).
                    nc.gpsimd.dma_start(ys[bp], outs[bp][:])
                else:
                    # Final plane: two SWDGE stores (engine-spread; a
                    # HWDGE store from sync/scalar lands on ONE SDMA
                    # engine at ~22 GB/s), each issued right after its
                    # half's drain so the ACT half streams while the
                    # DVE half still drains.
                    nc.gpsimd.dma_start(
                        ys[bp][:, 0 : 2 * W], outs[bp][:, 0 : 2 * W]
                    )
                    nc.gpsimd.dma_start(
                        ys[bp][:, 2 * W : 4 * W], outs[bp][:, 2 * W : 4 * W]
                    )

    nc.compile()
    return nc


_CACHE: dict = {}


def _get_nc(scale: float):
    key = (scale, USE_FP8, XSPLIT)
    if key not in _CACHE:
        _CACHE[key] = _build_nc(scale)
    return _CACHE[key]


def kernel(x: np.ndarray, weight: np.ndarray, _trace: bool = False):
    x = np.ascontiguousarray(x, dtype=np.float32)
    w = np.asarray(weight, dtype=np.float32).reshape(KTAP, KTAP)
    scale = float(w[KPAD, KPAD])  # 1/25 for the box kernel

    # Swizzle [plane, row, col] -> [plane, p, (kb, col)] with
    # row = kb*128 + p, so each SBUF partition line is one contiguous
    # DRAM chunk (maximal DMA descriptors); then group 4 planes per
    # partition line ([group, p, (plane-in-group, kb, col)]) so groups
    # load as single 1 MB DMAs.
    xs = (
        x.reshape(PLANES_TOTAL, 4, P, W)
        .transpose(0, 2, 1, 3)
        .reshape(PLANES_TOTAL // 4, 4, P, 4 * W)
        .transpose(0, 2, 1, 3)
        .reshape(PLANES_TOTAL // 4, P, 4 * 4 * W)
        .astype(NP_A_DT)
    )
    banda = _band_host(NP_A_DT)
    bandb = _band_host(NP_IO_DT)

    nc = _get_nc(scale)
    in_maps = [
        {
            "xs": xs[k * 4 : (k + 1) * 4],
            "banda": banda,
            "bandb": bandb,
        }
        for k in range(N_CORES)
    ]
    res = run_bass_kernel_spmd(nc, in_maps, list(range(N_CORES)), trace=_trace)
    out = np.concatenate(
        [np.asarray(r["ys"], dtype=np.float32) for r in res.results], axis=0
    )
    if _trace:
        kernel.last_exec_time_ns = res.exec_time_ns
        kernel.last_result = res
    # Undo the swizzle: [plane, p, (kb, col)] -> [plane, kb*128+p, col].
    out = (
        out.reshape(PLANES_TOTAL, P, 4, W)
        .transpose(0, 2, 1, 3)
        .reshape(16, 8, H, W)
    )
    return out


# revision 18
# speedup vs baseline: 1.0204x; 1.0204x over previous
"""Depthwise 5x5 box filter (stride 1, 'same' zero padding) on TRN2.

Input x: (16, 8, 512, 512) f32, weight: (1, 1, 5, 5) f32 (uniform box kernel).
Output: (16, 8, 512, 512) f32.

Strategy (v2)
-------------
Data-parallel over the 128 independent (n, c) planes: 16 planes per core
across 8 cores.  Per plane, the separable 5-tap box filter runs on the
TensorEngine as two "transposing" banded matmuls (pass A vertical, pass B
horizontal); each pass contracts over the partition dim so two passes
restore the original orientation with no explicit transposes.

v2 changes vs v1 (62 us):

  * Input is shipped as fp8 e3m4 (PE-native dtype): host-side RNE cast of
    the f32 input.  Quantization rel-L2 ~1.34e-2 << 2e-2 budget; halves
    input HBM traffic (4.2 MB/core) and SBUF footprint.  Pass A runs
    fp8 x fp8 (img x 0/1 band, exact), pass B fp16 x fp16 as before.
  * PSUM->SBUF drain restructured: each pass accumulates into ONE
    4-bank [128, 2048] PSUM tile, drained by exactly TWO ops: ACT takes
    cols [0:XSPLIT], DVE takes [XSPLIT:2048] (XSPLIT=1088 balances
    ACT@1.2GHz+~260ns/op against DVE@0.96GHz+~150ns/op at ~2.3us/plane
    per engine -- v1's 4-single-on-ACT split ran ACT at 2.76us/plane).
    The straddling drains still free bank 0..2 early enough for the
    next plane's matmuls (pipeline period ~2.4us > drain-op 1.2us).
  * Steady walls per plane: drains ~2.3us (ACT and DVE each), PE ~2.2us,
    DMA (256KB in + 512KB out)/420GB/s ~1.8us.
  * Tail: the last plane's stores are issued from ACT (HWDGE, right
    after its own B-drain), gpsimd and sync in parallel, in quarters,
    instead of 4 serialized ~650ns gpsimd issues.

Engine layout: PE interleaves pass A of plane p with pass B of plane p-1
(software pipeline, LAG=1).  32+8 warm-up matmuls lift the HAM clock gate
(1.2 -> 2.4 GHz) during the framework preamble's dead window.
"""

import os
from contextlib import ExitStack

import ml_dtypes
import numpy as np

import concourse.bacc as bacc
import concourse.tile as tile
from concourse import mybir
from concourse.bass_utils import run_bass_kernel_spmd

N_CORES = 8
PLANES_TOTAL = 128  # 16 batch * 8 channels
PLANES_PER_CORE = PLANES_TOTAL // N_CORES  # 16
H = W = 512
P = 128  # partitions / K-block
NB = P + 4  # band matrix columns
KTAP = 5
KPAD = 2

USE_FP8 = os.environ.get("BOXF_FP8", "1") == "1"
# PSUM drain split: ACT takes banks 0-1 (cols 0:1024) of each pass, DVE
# banks 2-3.  Must be (a) bank-aligned (ScalarE+VectorE may not touch the
# same PSUM bank concurrently) and (b) SEPARATE TILES (the tile framework
# serializes two engine-readers of one PSUM tile even on disjoint banks).
XSPLIT = 2 * W

MM_DT = mybir.dt.float16
NP_IO_DT = np.float16
A_DT = mybir.dt.float8e3 if USE_FP8 else mybir.dt.float16
NP_A_DT = ml_dtypes.float8_e3m4 if USE_FP8 else np.float16

# Per PSUM bank (one 512-wide output window) the 4 K-block matmuls write
# overlapping band windows; the first (start=True) clears the whole-bank
# pending-zero region, and subsequent matmuls accumulate where written /
# overwrite where pending, per-element (PSUM has_written semantics).
# (kb, out_lo, out_hi, band_lo, band_hi, start)
BANK_PLAN = [
    (0, 0, 130, 2, 132, True),
    (1, 126, 258, 0, 132, False),
    (2, 254, 386, 0, 132, False),
    (3, 382, 512, 0, 130, False),
]


def _band_host(np_dt) -> np.ndarray:
    """B[p, j] = 1.0 iff 0 <= j - p <= 4, shape [128, 132]."""
    b = np.zeros((P, NB), dtype=np.float32)
    for p in range(P):
        b[p, p : p + KTAP] = 1.0
    return b.astype(np_dt)


def _emit_bank(nc, ps_bank, band, lhsT_of, last_bank):
    for i, (kb, o0, o1, b0, b1, start) in enumerate(BANK_PLAN):
        nc.tensor.matmul(
            ps_bank[:, o0:o1],
            lhsT_of(kb),
            band[:, b0:b1],
            start=start,
            stop=(last_bank and i == len(BANK_PLAN) - 1),
        )


def _build_nc(scale: float):
    nc = bacc.Bacc("TRN2", num_devices=N_CORES, num_swdge_queues=4)
    # xs/ys live in DRAM pre-swizzled by the host to match the SBUF
    # partition-line layout exactly: element [pl, p, kb*W + w] is plane
    # pl's pixel (row kb*128 + p, col w).  Each partition line is one
    # contiguous DRAM chunk (2 KB fp8 in / 4 KB fp16 out) so every DMA
    # descriptor is maximal.
    # xs is stored as 4 groups of 4 planes, group-major then partition:
    # xs[g][p, k*2048 + c] is plane (4g+k)'s partition-p line.  Groups
    # 1..3 load with ONE 1 MB DMA each (8 KB per partition line).
    xs = nc.declare_dram_parameter(
        "xs", [4, P, 4 * 4 * W], A_DT, isOutput=False
    )
    banda_d = nc.declare_dram_parameter("banda", [P, NB], A_DT, isOutput=False)
    bandb_d = nc.declare_dram_parameter("bandb", [P, NB], MM_DT, isOutput=False)
    ys = nc.declare_dram_parameter(
        "ys", [PLANES_PER_CORE, P, 4 * W], MM_DT, isOutput=True
    )

    with ExitStack() as ctx:
        tc = ctx.enter_context(tile.TileContext(nc))
        const_pool = ctx.enter_context(tc.tile_pool(name="const", bufs=1))
        img_pool = ctx.enter_context(tc.tile_pool(name="img", bufs=4))
        gimg_pool = ctx.enter_context(tc.tile_pool(name="gimg", bufs=3))
        # 10-deep mid/out rotation: shallow pools put plane p's drains
        # behind plane p-k's consumers (cross-engine WAR stalls).
        mid_pool = ctx.enter_context(tc.tile_pool(name="mid", bufs=10))
        out_pool = ctx.enter_context(tc.tile_pool(name="out", bufs=10))
        psa_lo_pool = ctx.enter_context(tc.tile_pool(name="psal", bufs=1, space="PSUM"))
        psa_hi_pool = ctx.enter_context(tc.tile_pool(name="psah", bufs=1, space="PSUM"))
        psb_lo_pool = ctx.enter_context(tc.tile_pool(name="psbl", bufs=1, space="PSUM"))
        psb_hi_pool = ctx.enter_context(tc.tile_pool(name="psbh", bufs=1, space="PSUM"))

        banda = const_pool.tile([P, NB], A_DT, tag="banda")
        bandb = const_pool.tile([P, NB], MM_DT, tag="bandb")
        # Band must be the first Sync issue: on the ACT ring it queues
        # behind the auto-inserted ACT_TABLE_LOAD and delays every
        # pass-A matmul by ~2 us.
        nc.sync.dma_start(banda[:], banda_d[:])
        nc.sync.dma_start(bandb[:], bandb_d[:])

        # PE warm-up: the HAM clock gate holds the PE at 1.2 GHz until
        # it has been busy for a ~3.4 us activity window.  The first
        # input's DMA completion lands ~3.5 us after the preamble ends,
        # so burn that dead window on dummy matmuls over a memset
        # scratch tile -- the first real pass then runs at 2.4 GHz.
        warm_src = const_pool.tile([P, P], MM_DT, tag="warm")
        nc.gpsimd.memset(warm_src[:], 0)
        warm_ps = psa_lo_pool.tile(
            [P, 2 * W], mybir.dt.float32, tag="psal", name="warm"
        )
        for _ in range(32):
            nc.tensor.matmul(
                warm_ps[:, 0:P], warm_src[:], warm_src[:], start=True, stop=True
            )

        # All input DMAs up-front: the Sync/HWDGE ring issues them
        # back-to-back so the input stream saturates HBM from the start.
        # SBUF holds all 16 fp8 planes (32 KB/partition).  Plane 0 is
        # split (quarter + quarter + half) so the first pass-A matmul
        # gates on a 64 KB piece; planes 4..15 are grouped 4-at-a-time
        # into one DMA each (fewer issue slots and completion
        # semaphores; their data still arrives well ahead of use).
        imgs = {}  # pl -> (tile, base column)
        for pl in range(4):
            img = img_pool.tile([P, 4 * W], A_DT, tag="img", name=f"img{pl}")
            if pl == 0:
                # img free dim is (wb, kb, c'): the first half is exactly
                # pass-A banks 0-1's lhsT data, so bank-0 matmuls gate on
                # a 128 KB piece and banks 2-3 on the second piece.
                for c0, c1 in ((0, 2 * W), (2 * W, 4 * W)):
                    nc.sync.dma_start(img[:, c0:c1], xs[0][:, c0:c1])
            else:
                b = pl * 4 * W
                nc.sync.dma_start(img[:], xs[0][:, b : b + 4 * W])
            imgs[pl] = (img, 0)
        for g in range(1, 4):
            gimg = gimg_pool.tile(
                [P, 4 * 4 * W], A_DT, tag="gimg", name=f"img_g{g}"
            )
            nc.sync.dma_start(gimg[:], xs[g])
            for k in range(4):
                imgs[4 * g + k] = (gimg, k * 4 * W)

        # Software pipeline, LAG=1: PE runs pass A of plane pl then pass
        # B of plane pl-1.  Each pass accumulates into one 4-bank PSUM
        # tile, drained by exactly two ops (ACT cols [0:XSPLIT], DVE the
        # rest) -- minimal per-op overhead at balanced engine load.
        LAG = 1
        mids, outs = {}, {}
        for pl in range(PLANES_PER_CORE + LAG):
            bp = pl - LAG
            last_plane = bp == PLANES_PER_CORE - 1
            if pl < PLANES_PER_CORE:
                psa_lo = psa_lo_pool.tile(
                    [P, 2 * W], mybir.dt.float32, tag="psal", name=f"psal{pl}"
                )
                psa_hi = psa_hi_pool.tile(
                    [P, 2 * W], mybir.dt.float32, tag="psah", name=f"psah{pl}"
                )
                img, ib = imgs[pl]
                for wb in range(4):
                    ps = psa_lo if wb < 2 else psa_hi
                    o = (wb % 2) * W
                    # img free-dim layout is (wb, kb, c'): bank wb's four
                    # K-block lhsT slices are one contiguous 512-col run.
                    _emit_bank(
                        nc,
                        ps[:, o : o + W],
                        banda,
                        lambda kb, wb=wb: img[
                            :, ib + wb * W + kb * P : ib + wb * W + (kb + 1) * P
                        ],
                        last_bank=(wb % 2 == 1),
                    )
                mids[pl] = mid_pool.tile([P, 4 * W], MM_DT, tag="mid", name=f"mid{pl}")
                # Pass-A drain: plain downcast copies, ACT lo / DVE hi.
                nc.scalar.copy(mids[pl][:, 0:XSPLIT], psa_lo[:])
                nc.vector.tensor_copy(mids[pl][:, XSPLIT:], psa_hi[:])
            if pl == 0:
                # Second warm-up burst: fills the PE idle while the
                # first input's receipt lands, keeping the HAM activity
                # window busy through the pipeline fill.  Targets the
                # psb tile, which B(0) overwrites (start=True) after.
                fill_ps = psb_lo_pool.tile(
                    [P, 2 * W], mybir.dt.float32, tag="psbl", name="warmfill"
                )
                for _ in range(8):
                    nc.tensor.matmul(
                        fill_ps[:, 0:P], warm_src[:], warm_src[:],
                        start=True, stop=True,
                    )
            if bp >= 0:
                psb_lo = psb_lo_pool.tile(
                    [P, 2 * W], mybir.dt.float32, tag="psbl", name=f"psbl{bp}"
                )
                psb_hi = psb_hi_pool.tile(
                    [P, 2 * W], mybir.dt.float32, tag="psbh", name=f"psbh{bp}"
                )
                outs[bp] = out_pool.tile(
                    [P, 4 * W], MM_DT, tag="out", name=f"out{bp}"
                )
                mid = mids[bp]
                for wb in range(4):
                    ps = psb_lo if wb < 2 else psb_hi
                    o = (wb % 2) * W
                    _emit_bank(
                        nc,
                        ps[:, o : o + W],
                        bandb,
                        lambda kb, wb=wb: mid[
                            :, kb * W + wb * P : kb * W + (wb + 1) * P
                        ],
                        last_bank=(wb % 2 == 1),
                    )
                # Pass-B drain: fold the 1/25 scale into the downcast.
                nc.scalar.mul(outs[bp][:, 0:XSPLIT], psb_lo[:], scale)
                nc.vector.tensor_scalar_mul(
                    outs[bp][:, XSPLIT:], psb_hi[:], scale
                )
                if not last_plane:
                    # One full-plane output DMA on SWDGE (waits both
                    # drains via region deps).
                    nc.gpsimd.dma_start(ys[bp], outs[bp][:])
                else:
                    # Final plane: two SWDGE stores (engine-spread; a
                    # HWDGE store from sync/scalar lands on ONE SDMA
                    # engine at ~22 GB/s), each issued right after its
                    # half's drain so the ACT half streams while the
                    # DVE half still drains.
                    nc.gpsimd.dma_start(
                        ys[bp][:, 0 : 2 * W], outs[bp][:, 0 : 2 * W]
                    )
                    nc.gpsimd.dma_start(
                        ys[bp][:, 2 * W : 4 * W], outs[bp][:, 2 * W : 4 * W]
                    )

    nc.compile()
    return nc


_CACHE: dict = {}


def _get_nc(scale: float):
    key = (scale, USE_FP8, XSPLIT)
    if key not in _CACHE:
        _CACHE[key] = _build_nc(scale)
    return _CACHE[key]


def kernel(x: np.ndarray, weight: np.ndarray, _trace: bool = False):
    x = np.ascontiguousarray(x, dtype=np.float32)
    w = np.asarray(weight, dtype=np.float32).reshape(KTAP, KTAP)
    scale = float(w[KPAD, KPAD])  # 1/25 for the box kernel

    # Swizzle [plane, row, col] -> [plane, p, (wb, kb, c')] with
    # row = kb*128 + p and col = wb*128 + c'.  Each partition line is one
    # contiguous DRAM chunk, AND each 512-col piece is exactly one
    # pass-A bank's lhsT data (so partial loads gate banks).  Then group
    # 4 planes per partition line so groups load as single 1 MB DMAs.
    xs = (
        x.reshape(PLANES_TOTAL, 4, P, 4, P)
        .transpose(0, 2, 3, 1, 4)
        .reshape(PLANES_TOTAL // 4, 4, P, 4 * W)
        .transpose(0, 2, 1, 3)
        .reshape(PLANES_TOTAL // 4, P, 4 * 4 * W)
        .astype(NP_A_DT)
    )
    banda = _band_host(NP_A_DT)
    bandb = _band_host(NP_IO_DT)

    nc = _get_nc(scale)
    in_maps = [
        {
            "xs": xs[k * 4 : (k + 1) * 4],
            "banda": banda,
            "bandb": bandb,
        }
        for k in range(N_CORES)
    ]
    res = run_bass_kernel_spmd(nc, in_maps, list(range(N_CORES)), trace=_trace)
    out = np.concatenate(
        [np.asarray(r["ys"], dtype=np.float32) for r in res.results], axis=0
    )
    if _trace:
        kernel.last_exec_time_ns = res.exec_time_ns
        kernel.last_result = res
    # Undo the swizzle: [plane, p, (kb, col)] -> [plane, kb*128+p, col].
    out = (
        out.reshape(PLANES_TOTAL, P, 4, W)
        .transpose(0, 2, 1, 3)
        .reshape(16, 8, H, W)
    )
    return out


# revision 20
# speedup vs baseline: 1.0266x; 1.0061x over previous
"""Depthwise 5x5 box filter (stride 1, 'same' zero padding) on TRN2.

Input x: (16, 8, 512, 512) f32, weight: (1, 1, 5, 5) f32 (uniform box kernel).
Output: (16, 8, 512, 512) f32.

Strategy (v2)
-------------
Data-parallel over the 128 independent (n, c) planes: 16 planes per core
across 8 cores.  Per plane, the separable 5-tap box filter runs on the
TensorEngine as two "transposing" banded matmuls (pass A vertical, pass B
horizontal); each pass contracts over the partition dim so two passes
restore the original orientation with no explicit transposes.

v2 changes vs v1 (62 us):

  * Input is shipped as fp8 e3m4 (PE-native dtype): host-side RNE cast of
    the f32 input.  Quantization rel-L2 ~1.34e-2 << 2e-2 budget; halves
    input HBM traffic (4.2 MB/core) and SBUF footprint.  Pass A runs
    fp8 x fp8 (img x 0/1 band, exact), pass B fp16 x fp16 as before.
  * PSUM->SBUF drain restructured: each pass accumulates into ONE
    4-bank [128, 2048] PSUM tile, drained by exactly TWO ops: ACT takes
    cols [0:XSPLIT], DVE takes [XSPLIT:2048] (XSPLIT=1088 balances
    ACT@1.2GHz+~260ns/op against DVE@0.96GHz+~150ns/op at ~2.3us/plane
    per engine -- v1's 4-single-on-ACT split ran ACT at 2.76us/plane).
    The straddling drains still free bank 0..2 early enough for the
    next plane's matmuls (pipeline period ~2.4us > drain-op 1.2us).
  * Steady walls per plane: drains ~2.3us (ACT and DVE each), PE ~2.2us,
    DMA (256KB in + 512KB out)/420GB/s ~1.8us.
  * Tail: the last plane's stores are issued from ACT (HWDGE, right
    after its own B-drain), gpsimd and sync in parallel, in quarters,
    instead of 4 serialized ~650ns gpsimd issues.

Engine layout: PE interleaves pass A of plane p with pass B of plane p-1
(software pipeline, LAG=1).  32+8 warm-up matmuls lift the HAM clock gate
(1.2 -> 2.4 GHz) during the framework preamble's dead window.
"""

import os
from contextlib import ExitStack

import ml_dtypes
import numpy as np

import concourse.bacc as bacc
import concourse.tile as tile
from concourse import mybir
from concourse.bass_utils import run_bass_kernel_spmd

N_CORES = 8
PLANES_TOTAL = 128  # 16 batch * 8 channels
PLANES_PER_CORE = PLANES_TOTAL // N_CORES  # 16
H = W = 512
P = 128  # partitions / K-block
NB = P + 4  # band matrix columns
KTAP = 5
KPAD = 2

USE_FP8 = os.environ.get("BOXF_FP8", "1") == "1"
# PSUM drain split: ACT takes banks 0-1 (cols 0:1024) of each pass, DVE
# banks 2-3.  Must be (a) bank-aligned (ScalarE+VectorE may not touch the
# same PSUM bank concurrently) and (b) SEPARATE TILES (the tile framework
# serializes two engine-readers of one PSUM tile even on disjoint banks).
XSPLIT = 2 * W

MM_DT = mybir.dt.float16
NP_IO_DT = np.float16
A_DT = mybir.dt.float8e3 if USE_FP8 else mybir.dt.float16
NP_A_DT = ml_dtypes.float8_e3m4 if USE_FP8 else np.float16

# Per PSUM bank (one 512-wide output window) the 4 K-block matmuls write
# overlapping band windows; the first (start=True) clears the whole-bank
# pending-zero region, and subsequent matmuls accumulate where written /
# overwrite where pending, per-element (PSUM has_written semantics).
# (kb, out_lo, out_hi, band_lo, band_hi, start)
BANK_PLAN = [
    (0, 0, 130, 2, 132, True),
    (1, 126, 258, 0, 132, False),
    (2, 254, 386, 0, 132, False),
    (3, 382, 512, 0, 130, False),
]


def _band_host(np_dt) -> np.ndarray:
    """B[p, j] = 1.0 iff 0 <= j - p <= 4, shape [128, 132]."""
    b = np.zeros((P, NB), dtype=np.float32)
    for p in range(P):
        b[p, p : p + KTAP] = 1.0
    return b.astype(np_dt)


def _emit_bank(nc, ps_bank, band, lhsT_of, last_bank):
    for i, (kb, o0, o1, b0, b1, start) in enumerate(BANK_PLAN):
        nc.tensor.matmul(
            ps_bank[:, o0:o1],
            lhsT_of(kb),
            band[:, b0:b1],
            start=start,
            stop=(last_bank and i == len(BANK_PLAN) - 1),
        )


def _build_nc(scale: float):
    nc = bacc.Bacc("TRN2", num_devices=N_CORES, num_swdge_queues=4)
    # xs/ys live in DRAM pre-swizzled by the host to match the SBUF
    # partition-line layout exactly: element [pl, p, kb*W + w] is plane
    # pl's pixel (row kb*128 + p, col w).  Each partition line is one
    # contiguous DRAM chunk (2 KB fp8 in / 4 KB fp16 out) so every DMA
    # descriptor is maximal.
    # xs is stored as 4 groups of 4 planes, group-major then partition:
    # xs[g][p, k*2048 + c] is plane (4g+k)'s partition-p line.  Groups
    # 1..3 load with ONE 1 MB DMA each (8 KB per partition line).
    xs = nc.declare_dram_parameter(
        "xs", [4, P, 4 * 4 * W], A_DT, isOutput=False
    )
    banda_d = nc.declare_dram_parameter("banda", [P, NB], A_DT, isOutput=False)
    bandb_d = nc.declare_dram_parameter("bandb", [P, NB], MM_DT, isOutput=False)
    ys = nc.declare_dram_parameter(
        "ys", [PLANES_PER_CORE, P, 4 * W], MM_DT, isOutput=True
    )

    with ExitStack() as ctx:
        tc = ctx.enter_context(tile.TileContext(nc))
        const_pool = ctx.enter_context(tc.tile_pool(name="const", bufs=1))
        img_pool = ctx.enter_context(tc.tile_pool(name="img", bufs=4))
        gimg_pool = ctx.enter_context(tc.tile_pool(name="gimg", bufs=3))
        # 10-deep mid/out rotation: shallow pools put plane p's drains
        # behind plane p-k's consumers (cross-engine WAR stalls).
        mid_pool = ctx.enter_context(tc.tile_pool(name="mid", bufs=10))
        out_pool = ctx.enter_context(tc.tile_pool(name="out", bufs=10))
        psa_lo_pool = ctx.enter_context(tc.tile_pool(name="psal", bufs=1, space="PSUM"))
        psa_hi_pool = ctx.enter_context(tc.tile_pool(name="psah", bufs=1, space="PSUM"))
        psb_lo_pool = ctx.enter_context(tc.tile_pool(name="psbl", bufs=1, space="PSUM"))
        psb_hi_pool = ctx.enter_context(tc.tile_pool(name="psbh", bufs=1, space="PSUM"))

        banda = const_pool.tile([P, NB], A_DT, tag="banda")
        bandb = const_pool.tile([P, NB], MM_DT, tag="bandb")
        # PE warm-up: the HAM clock gate holds the PE at 1.2 GHz until
        # it has been busy for a ~3.4 us activity window.  The first
        # input's DMA completion lands ~3.5 us after the preamble ends,
        # so burn that dead window on dummy matmuls over a memset
        # scratch tile -- the first real pass then runs at 2.4 GHz.
        warm_src = const_pool.tile([P, P], MM_DT, tag="warm")
        nc.gpsimd.memset(warm_src[:], 0)
        # Bands go out on GpSimd/SWDGE (idle during the fill) so the
        # Sync/HWDGE ring's first issue slots belong to the input
        # pieces -- every input receipt lands ~1.3 us earlier.
        nc.gpsimd.dma_start(banda[:], banda_d[:])
        nc.gpsimd.dma_start(bandb[:], bandb_d[:])
        warm_ps = psa_lo_pool.tile(
            [P, 2 * W], mybir.dt.float32, tag="psal", name="warm"
        )
        for _ in range(32):
            nc.tensor.matmul(
                warm_ps[:, 0:P], warm_src[:], warm_src[:], start=True, stop=True
            )

        # All input DMAs up-front: the Sync/HWDGE ring issues them
        # back-to-back so the input stream saturates HBM from the start.
        # SBUF holds all 16 fp8 planes (32 KB/partition).  Plane 0 is
        # split (quarter + quarter + half) so the first pass-A matmul
        # gates on a 64 KB piece; planes 4..15 are grouped 4-at-a-time
        # into one DMA each (fewer issue slots and completion
        # semaphores; their data still arrives well ahead of use).
        imgs = {}  # pl -> (tile, base column)
        for pl in range(4):
            img = img_pool.tile([P, 4 * W], A_DT, tag="img", name=f"img{pl}")
            if pl == 0:
                # img free dim is (wb, kb, c'): the first half is exactly
                # pass-A banks 0-1's lhsT data, so bank-0 matmuls gate on
                # a 128 KB piece and banks 2-3 on the second piece.
                for c0, c1 in ((0, 2 * W), (2 * W, 4 * W)):
                    nc.sync.dma_start(img[:, c0:c1], xs[0][:, c0:c1])
            else:
                b = pl * 4 * W
                nc.sync.dma_start(img[:], xs[0][:, b : b + 4 * W])
            imgs[pl] = (img, 0)
        for g in range(1, 4):
            gimg = gimg_pool.tile(
                [P, 4 * 4 * W], A_DT, tag="gimg", name=f"img_g{g}"
            )
            nc.sync.dma_start(gimg[:], xs[g])
            for k in range(4):
                imgs[4 * g + k] = (gimg, k * 4 * W)

        # Software pipeline, LAG=1: PE runs pass A of plane pl then pass
        # B of plane pl-1.  Each pass accumulates into one 4-bank PSUM
        # tile, drained by exactly two ops (ACT cols [0:XSPLIT], DVE the
        # rest) -- minimal per-op overhead at balanced engine load.
        LAG = 1
        mids, outs = {}, {}
        for pl in range(PLANES_PER_CORE + LAG):
            bp = pl - LAG
            last_plane = bp == PLANES_PER_CORE - 1
            if pl < PLANES_PER_CORE:
                psa_lo = psa_lo_pool.tile(
                    [P, 2 * W], mybir.dt.float32, tag="psal", name=f"psal{pl}"
                )
                psa_hi = psa_hi_pool.tile(
                    [P, 2 * W], mybir.dt.float32, tag="psah", name=f"psah{pl}"
                )
                img, ib = imgs[pl]
                for wb in range(4):
                    ps = psa_lo if wb < 2 else psa_hi
                    o = (wb % 2) * W
                    # img free-dim layout is (wb, kb, c'): bank wb's four
                    # K-block lhsT slices are one contiguous 512-col run.
                    _emit_bank(
                        nc,
                        ps[:, o : o + W],
                        banda,
                        lambda kb, wb=wb: img[
                            :, ib + wb * W + kb * P : ib + wb * W + (kb + 1) * P
                        ],
                        last_bank=(wb % 2 == 1),
                    )
                mids[pl] = mid_pool.tile([P, 4 * W], MM_DT, tag="mid", name=f"mid{pl}")
                # Pass-A drain: plain downcast copies, ACT lo / DVE hi.
                nc.scalar.copy(mids[pl][:, 0:XSPLIT], psa_lo[:])
                nc.vector.tensor_copy(mids[pl][:, XSPLIT:], psa_hi[:])
            if pl == 0:
                # Second warm-up burst: fills the PE idle while the
                # first input's receipt lands, keeping the HAM activity
                # window busy through the pipeline fill.  Targets the
                # psb tile, which B(0) overwrites (start=True) after.
                fill_ps = psb_lo_pool.tile(
                    [P, 2 * W], mybir.dt.float32, tag="psbl", name="warmfill"
                )
                for _ in range(8):
                    nc.tensor.matmul(
                        fill_ps[:, 0:P], warm_src[:], warm_src[:],
                        start=True, stop=True,
                    )
            if bp >= 0:
                psb_lo = psb_lo_pool.tile(
                    [P, 2 * W], mybir.dt.float32, tag="psbl", name=f"psbl{bp}"
                )
                psb_hi = psb_hi_pool.tile(
                    [P, 2 * W], mybir.dt.float32, tag="psbh", name=f"psbh{bp}"
                )
                outs[bp] = out_pool.tile(
                    [P, 4 * W], MM_DT, tag="out", name=f"out{bp}"
                )
                mid = mids[bp]
                for wb in range(4):
                    ps = psb_lo if wb < 2 else psb_hi
                    o = (wb % 2) * W
                    _emit_bank(
                        nc,
                        ps[:, o : o + W],
                        bandb,
                        lambda kb, wb=wb: mid[
                            :, kb * W + wb * P : kb * W + (wb + 1) * P
                        ],
                        last_bank=(wb % 2 == 1),
                    )
                # Pass-B drain: fold the 1/25 scale into the downcast.
                nc.scalar.mul(outs[bp][:, 0:XSPLIT], psb_lo[:], scale)
                nc.vector.tensor_scalar_mul(
                    outs[bp][:, XSPLIT:], psb_hi[:], scale
                )
                if bp < PLANES_PER_CORE - 2:
                    # One full-plane output DMA on SWDGE (waits both
                    # drains via region deps).
                    nc.gpsimd.dma_start(ys[bp], outs[bp][:])
                else:
                    # Last two planes: two half-plane SWDGE stores each
                    # (a HWDGE store from sync/scalar lands on ONE SDMA
                    # engine at ~22 GB/s), the ACT half issued while the
                    # DVE half still drains -- shortens the final
                    # output backlog.
                    nc.gpsimd.dma_start(
                        ys[bp][:, 0 : 2 * W], outs[bp][:, 0 : 2 * W]
                    )
                    nc.gpsimd.dma_start(
                        ys[bp][:, 2 * W : 4 * W], outs[bp][:, 2 * W : 4 * W]
                    )

    nc.compile()
    return nc


_CACHE: dict = {}


def _get_nc(scale: float):
    key = (scale, USE_FP8, XSPLIT)
    if key not in _CACHE:
        _CACHE[key] = _build_nc(scale)
    return _CACHE[key]


def kernel(x: np.ndarray, weight: np.ndarray, _trace: bool = False):
    x = np.ascontiguousarray(x, dtype=np.float32)
    w = np.asarray(weight, dtype=np.float32).reshape(KTAP, KTAP)
    scale = float(w[KPAD, KPAD])  # 1/25 for the box kernel

    # Swizzle [plane, row, col] -> [plane, p, (wb, kb, c')] with
    # row = kb*128 + p and col = wb*128 + c'.  Each partition line is one
    # contiguous DRAM chunk, AND each 512-col piece is exactly one
    # pass-A bank's lhsT data (so partial loads gate banks).  Then group
    # 4 planes per partition line so groups load as single 1 MB DMAs.
    xs = (
        x.reshape(PLANES_TOTAL, 4, P, 4, P)
        .transpose(0, 2, 3, 1, 4)
        .reshape(PLANES_TOTAL // 4, 4, P, 4 * W)
        .transpose(0, 2, 1, 3)
        .reshape(PLANES_TOTAL // 4, P, 4 * 4 * W)
        .astype(NP_A_DT)
    )
    banda = _band_host(NP_A_DT)
    bandb = _band_host(NP_IO_DT)

    nc = _get_nc(scale)
    in_maps = [
        {
            "xs": xs[k * 4 : (k + 1) * 4],
            "banda": banda,
            "bandb": bandb,
        }
        for k in range(N_CORES)
    ]
    res = run_bass_kernel_spmd(nc, in_maps, list(range(N_CORES)), trace=_trace)
    out = np.concatenate(
        [np.asarray(r["ys"], dtype=np.float32) for r in res.results], axis=0
    )
    if _trace:
        kernel.last_exec_time_ns = res.exec_time_ns
        kernel.last_result = res
    # Undo the swizzle: [plane, p, (kb, col)] -> [plane, kb*128+p, col].
    out = (
        out.reshape(PLANES_TOTAL, P, 4, W)
        .transpose(0, 2, 1, 3)
        .reshape(16, 8, H, W)
    )
    return out
